# revision 2
# baseline (speedup 1.0000x reference)
"""Trainium2 Bass kernel for nn_ElementRelationships.

Math: out[b,t,n,f] = input[b,t,n,f] * 1.1  if n < batch_set_size[b,t] else 0.

Pure data parallel over B (32) across 8 cores -> 4 batches/core.
Per core: x shard [4,64,128,256] f32 = 32 MiB in + 32 MiB out.

Device layout: flatten bt = (b*64+t) in [0,256). View the shard as
[a=2, p=128, s=NSPLIT, m] where bt = a*128 + p and each (a,s) tile is
[128 partitions, m] with fully contiguous per-partition DMA descriptors.
The ragged mask (with the 1.1 scale baked in) is computed on host as a
[128, 256] f32 tile (128 KiB): mask_sb[p, a*128 + n] = 1.1*(n < set[a*128+p]).
Each data tile is multiplied in place on DVE by a step-0-broadcast slice
of the mask (per (bt, n) scalar broadcast over f=256).
"""

import numpy as np

from contextlib import ExitStack

import concourse.bass as bass
import concourse.tile as tile
from concourse import bacc, mybir
from concourse import bass_utils

B, T, N, F = 32, 64, 128, 256
SCALE = 1.1  # ALPHA + BETA
N_CORES = 8
BPC = B // N_CORES            # batches per core = 4
BT = BPC * T                  # 256 flattened (b,t) rows per core
A = BT // 128                 # 2 partition-groups of bt
NSPLIT = 4                    # tiles per (a) group
NPER = N // NSPLIT            # n-rows per tile = 32
M = NPER * F                  # free elems per partition per tile = 8192
BUFS = 4

_CACHE = {}


def _build():
    nc = bacc.Bacc(
        "TRN2",
        target_bir_lowering=False,
        debug=False,
        enable_asserts=False,
        num_devices=N_CORES,
    )
    x = nc.dram_tensor("x", [A, 128, NSPLIT, M], mybir.dt.float32,
                       kind="ExternalInput").ap()
    mask = nc.dram_tensor("mask", [128, A * N], mybir.dt.float32,
                          kind="ExternalInput").ap()
    y = nc.dram_tensor("y", [A, 128, NSPLIT, M], mybir.dt.float32,
                       kind="ExternalOutput").ap()

    with tile.TileContext(nc) as tc:
        with ExitStack() as ctx:
            mask_pool = ctx.enter_context(tc.tile_pool(name="maskp", bufs=1))
            pool = ctx.enter_context(tc.tile_pool(name="xp", bufs=BUFS))

            mask_sb = mask_pool.tile([128, A * N], mybir.dt.float32, name="mask_sb")
            nc.sync.dma_start(mask_sb[:], mask[:])

            for a in range(A):
                for s in range(NSPLIT):
                    t = pool.tile([128, M], mybir.dt.float32, name="xt")
                    nc.sync.dma_start(t[:], x[a, :, s, :])
                    t3 = t.rearrange("p (n f) -> p n f", f=F)
                    msl = mask_sb[:, a * N + s * NPER: a * N + (s + 1) * NPER]
                    nc.vector.tensor_mul(
                        t3, t3, msl.unsqueeze(2).broadcast_to((128, NPER, F))
                    )
                    nc.sync.dma_start(y[a, :, s, :], t[:])

    nc.compile()
    return nc


def _get_nc():
    if "nc" not in _CACHE:
        _CACHE["nc"] = _build()
    return _CACHE["nc"]


def _host_prep(input_tensor, batch_set_size):
    """Build per-core in_maps (x shard + mask tile)."""
    n_idx = np.arange(N, dtype=np.int64)
    # [B, T, N] f32 mask with scale baked in
    mfull = (n_idx[None, None, :] < np.asarray(batch_set_size)[:, :, None])
    mfull = mfull.astype(np.float32) * np.float32(SCALE)

    in_maps = []
    for i in range(N_CORES):
        xs = np.ascontiguousarray(input_tensor[i * BPC:(i + 1) * BPC])
        xs = xs.reshape(A, 128, NSPLIT, M)
        mf = mfull[i * BPC:(i + 1) * BPC].reshape(BT, N)          # [256, 128]
        mdev = np.ascontiguousarray(
            mf.reshape(A, 128, N).transpose(1, 0, 2).reshape(128, A * N)
        )
        in_maps.append({"x": xs, "mask": mdev})
    return in_maps


def kernel(input_tensor, batch_set_size):
    input_tensor = np.asarray(input_tensor, dtype=np.float32)
    nc = _get_nc()
    in_maps = _host_prep(input_tensor, batch_set_size)
    res = bass_utils.run_bass_kernel_spmd(
        nc, in_maps, core_ids=list(range(N_CORES))
    )
    outs = [
        r["y"].reshape(BPC, T, N, F) for r in res.results
    ]
    return np.concatenate(outs, axis=0)


# revision 4
# speedup vs baseline: 1.0422x; 1.0422x over previous
"""Trainium2 Bass kernel for nn_ElementRelationships.

Math: out[b,t,n,f] = input[b,t,n,f] * 1.1  if n < batch_set_size[b,t] else 0.

Pure data parallel over B (32) across 8 cores -> 4 batches/core.
Per core: x shard [4,64,128,256] f32 = 32 MiB in + 32 MiB out.

Device layout: flatten bt = (b*64+t) in [0,256). View the shard as
[a=2, p=128, s=NSPLIT, m] where bt = a*128 + p and each (a,s) tile is
[128 partitions, m] with fully contiguous per-partition DMA descriptors.
The ragged mask (with the 1.1 scale baked in) is computed on host as a
[128, 256] f32 tile (128 KiB): mask_sb[p, a*128 + n] = 1.1*(n < set[a*128+p]).
Each data tile is multiplied in place on DVE by a step-0-broadcast slice
of the mask (per (bt, n) scalar broadcast over f=256).
"""

import numpy as np

from contextlib import ExitStack

import concourse.bass as bass
import concourse.tile as tile
from concourse import bacc, mybir
from concourse import bass_utils

B, T, N, F = 32, 64, 128, 256
SCALE = 1.1  # ALPHA + BETA
N_CORES = 8
BPC = B // N_CORES            # batches per core = 4
BT = BPC * T                  # 256 flattened (b,t) rows per core
A = BT // 128                 # 2 partition-groups of bt
NSPLIT = 8                    # tiles per (a) group
NPER = N // NSPLIT            # n-rows per tile
M = NPER * F                  # free elems per partition per tile
BUFS = 8

_CACHE = {}


def _build():
    nc = bacc.Bacc(
        "TRN2",
        target_bir_lowering=False,
        debug=False,
        enable_asserts=False,
        num_devices=N_CORES,
    )
    x = nc.dram_tensor("x", [A, 128, NSPLIT, M], mybir.dt.float32,
                       kind="ExternalInput").ap()
    mask = nc.dram_tensor("mask", [128, A * N], mybir.dt.float32,
                          kind="ExternalInput").ap()
    y = nc.dram_tensor("y", [A, 128, NSPLIT, M], mybir.dt.float32,
                       kind="ExternalOutput").ap()

    with tile.TileContext(nc) as tc:
        with ExitStack() as ctx:
            mask_pool = ctx.enter_context(tc.tile_pool(name="maskp", bufs=1))
            pool = ctx.enter_context(tc.tile_pool(name="xp", bufs=BUFS))

            mask_sb = mask_pool.tile([128, A * N], mybir.dt.float32, name="mask_sb")
            nc.scalar.dma_start(mask_sb[:], mask[:])

            for a in range(A):
                for s in range(NSPLIT):
                    t = pool.tile([128, M], mybir.dt.float32, name="xt")
                    # loads on the SP HWDGE ring, stores on the ACT ring:
                    # a store stalled on its mul must not head-block loads.
                    nc.sync.dma_start(t[:], x[a, :, s, :])
                    t3 = t.rearrange("p (n f) -> p n f", f=F)
                    msl = mask_sb[:, a * N + s * NPER: a * N + (s + 1) * NPER]
                    nc.vector.tensor_mul(
                        t3, t3, msl.unsqueeze(2).broadcast_to((128, NPER, F))
                    )
                    nc.scalar.dma_start(y[a, :, s, :], t[:])

    nc.compile()
    return nc


def _get_nc():
    if "nc" not in _CACHE:
        _CACHE["nc"] = _build()
    return _CACHE["nc"]


def _host_prep(input_tensor, batch_set_size):
    """Build per-core in_maps (x shard + mask tile)."""
    n_idx = np.arange(N, dtype=np.int64)
    # [B, T, N] f32 mask with scale baked in
    mfull = (n_idx[None, None, :] < np.asarray(batch_set_size)[:, :, None])
    mfull = mfull.astype(np.float32) * np.float32(SCALE)

    in_maps = []
    for i in range(N_CORES):
        xs = np.ascontiguousarray(input_tensor[i * BPC:(i + 1) * BPC])
        xs = xs.reshape(A, 128, NSPLIT, M)
        mf = mfull[i * BPC:(i + 1) * BPC].reshape(BT, N)          # [256, 128]
        mdev = np.ascontiguousarray(
            mf.reshape(A, 128, N).transpose(1, 0, 2).reshape(128, A * N)
        )
        in_maps.append({"x": xs, "mask": mdev})
    return in_maps


def kernel(input_tensor, batch_set_size):
    input_tensor = np.asarray(input_tensor, dtype=np.float32)
    nc = _get_nc()
    in_maps = _host_prep(input_tensor, batch_set_size)
    res = bass_utils.run_bass_kernel_spmd(
        nc, in_maps, core_ids=list(range(N_CORES))
    )
    outs = [
        r["y"].reshape(BPC, T, N, F) for r in res.results
    ]
    return np.concatenate(outs, axis=0)


# revision 5
# speedup vs baseline: 1.2958x; 1.2433x over previous
"""Trainium2 Bass kernel for nn_ElementRelationships.

Math: out[b,t,n,f] = input[b,t,n,f] * 1.1  if n < batch_set_size[b,t] else 0.

Pure data parallel over B (32) across 8 cores -> 4 batches/core.
Per core: x shard [4,64,128,256] f32 = 32 MiB in + 32 MiB out dense.

Layout: flatten bt = (b*64+t) in [0,256) per core.  Host sorts the 256
rows by set_size (descending) and permutes x accordingly (inverted when
reassembling the output).  The shard is viewed as [a=2, p=128, s=NSPLIT, m]
where device row r = a*128 + p holds sorted-rank-r's block, and tile (a,s)
covers n in [s*NPER, (s+1)*NPER).  After sorting, the rows that need
chunk s form a partition prefix [0, K[a][s]) — so each tile is a single
partition-prefix DMA, and fully-masked rows are neither loaded nor
stored (ExternalOutput buffers are donated pre-zeroed by
run_bass_via_pjrt, so skipped rows read back as zeros).

The ragged mask (with the 1.1 scale baked in) rides along as a [128,256]
f32 input tile; each data tile is multiplied in place on DVE by a
step-0-broadcast slice of it (per (row, n) scalar broadcast over f=256).
K is rounded up to a multiple of 8 and maxed across cores, which only
adds rows whose mask is all-zero in that chunk (stored as zeros —
still exact).

Loads issue on the SP HWDGE ring, stores on the ACT ring, so a store
stalled on its mul never head-blocks later loads.
"""

import numpy as np

from contextlib import ExitStack

import concourse.bass as bass
import concourse.tile as tile
from concourse import bacc, mybir
from concourse import bass_utils

B, T, N, F = 32, 64, 128, 256
SCALE = 1.1  # ALPHA + BETA
N_CORES = 8
BPC = B // N_CORES            # batches per core = 4
BT = BPC * T                  # 256 flattened (b,t) rows per core
A = BT // 128                 # 2 partition-groups of rows
NSPLIT = 8                    # n-chunks per group
NPER = N // NSPLIT            # n-values per chunk
M = NPER * F                  # free elems per partition per tile
BUFS = 8
KQUANT = 8                    # round prefix counts up to a multiple of this

_CACHE = {}


def _build(kpat):
    """Build + compile the SPMD program for prefix-count pattern `kpat`
    (tuple of A*NSPLIT ints in [0,128], multiples of KQUANT)."""
    nc = bacc.Bacc(
        "TRN2",
        target_bir_lowering=False,
        debug=False,
        enable_asserts=False,
        num_devices=N_CORES,
    )
    x = nc.dram_tensor("x", [A, 128, NSPLIT, M], mybir.dt.float32,
                       kind="ExternalInput").ap()
    mask = nc.dram_tensor("mask", [128, A * N], mybir.dt.float32,
                          kind="ExternalInput").ap()
    y = nc.dram_tensor("y", [A, 128, NSPLIT, M], mybir.dt.float32,
                       kind="ExternalOutput").ap()

    with tile.TileContext(nc) as tc:
        with ExitStack() as ctx:
            mask_pool = ctx.enter_context(tc.tile_pool(name="maskp", bufs=1))
            pool = ctx.enter_context(tc.tile_pool(name="xp", bufs=BUFS))

            mask_sb = mask_pool.tile([128, A * N], mybir.dt.float32,
                                     name="mask_sb")
            nc.scalar.dma_start(mask_sb[:], mask[:])

            for a in range(A):
                for s in range(NSPLIT):
                    K = kpat[a * NSPLIT + s]
                    if K == 0:
                        continue
                    t = pool.tile([128, M], mybir.dt.float32, name="xt")
                    nc.sync.dma_start(t[0:K, :], x[a, 0:K, s, :])
                    t3 = t[0:K].rearrange("p (n f) -> p n f", f=F)
                    msl = mask_sb[0:K, a * N + s * NPER: a * N + (s + 1) * NPER]
                    nc.vector.tensor_mul(
                        t3, t3, msl.unsqueeze(2).broadcast_to((K, NPER, F))
                    )
                    nc.scalar.dma_start(y[a, 0:K, s, :], t[0:K, :])

    nc.compile()
    return nc


def _get_nc(kpat):
    if kpat not in _CACHE:
        _CACHE[kpat] = _build(kpat)
    return _CACHE[kpat]


def _host_prep(input_tensor, batch_set_size):
    """Sort rows per core, build in_maps and the global K pattern."""
    ss_all = np.asarray(batch_set_size).reshape(B, T).astype(np.int64)
    n_idx = np.arange(N, dtype=np.int64)

    in_maps = []
    perms = []
    kmat = np.zeros((N_CORES, A * NSPLIT), dtype=np.int64)
    for i in range(N_CORES):
        ss = ss_all[i * BPC:(i + 1) * BPC].reshape(BT)
        perm = np.argsort(-ss, kind="stable")
        perms.append(perm)
        ss_sorted = ss[perm]

        xs = np.asarray(input_tensor[i * BPC:(i + 1) * BPC],
                        dtype=np.float32).reshape(BT, N * F)
        x_dev = np.ascontiguousarray(xs[perm]).reshape(A, 128, NSPLIT, M)

        mrows = (n_idx[None, :] < ss_sorted[:, None]).astype(np.float32)
        mrows *= np.float32(SCALE)                           # [BT, N] sorted
        mdev = np.ascontiguousarray(
            mrows.reshape(A, 128, N).transpose(1, 0, 2).reshape(128, A * N)
        )
        in_maps.append({"x": x_dev, "mask": mdev})

        for a in range(A):
            g = ss_sorted[a * 128:(a + 1) * 128]
            for s in range(NSPLIT):
                kmat[i, a * NSPLIT + s] = int((g > s * NPER).sum())

    kmax = kmat.max(axis=0)
    kpat = tuple(
        int(min(128, -(-k // KQUANT) * KQUANT)) for k in kmax
    )
    return in_maps, perms, kpat


def kernel(input_tensor, batch_set_size):
    input_tensor = np.asarray(input_tensor, dtype=np.float32)
    in_maps, perms, kpat = _host_prep(input_tensor, batch_set_size)

    if all(k == 0 for k in kpat):
        return np.zeros((B, T, N, F), dtype=np.float32)

    nc = _get_nc(kpat)
    res = bass_utils.run_bass_kernel_spmd(
        nc, in_maps, core_ids=list(range(N_CORES))
    )
    out = np.empty((B, T, N, F), dtype=np.float32)
    for i in range(N_CORES):
        y_rows = res.results[i]["y"].reshape(BT, N * F)
        dst = out[i * BPC:(i + 1) * BPC].reshape(BT, N * F)
        dst[perms[i]] = y_rows
    return out


# revision 9
# speedup vs baseline: 1.3798x; 1.0648x over previous
"""Trainium2 Bass kernel for nn_ElementRelationships.

Math: out[b,t,n,f] = input[b,t,n,f] * 1.1  if n < batch_set_size[b,t] else 0.

Pure data parallel over B (32) across 8 cores -> 4 batches/core.
Per core: x shard [4,64,128,256] f32 = 32 MiB in + 32 MiB out dense.

Layout: flatten bt = (b*64+t) in [0,256) per core.  Host sorts the 256
rows by set_size (descending) and permutes x accordingly (inverted when
reassembling the output).  The shard is viewed as [a=2, p=128, s=NSPLIT, m]
where device row r = a*128 + p holds sorted-rank-r's block, and tile (a,s)
covers n in [s*NPER, (s+1)*NPER).  After sorting, the rows that need
chunk s form a partition prefix [0, K[a][s]) — so each tile is a single
partition-prefix DMA, and fully-masked rows are neither loaded nor
stored (ExternalOutput buffers are donated pre-zeroed by
run_bass_via_pjrt, so skipped rows read back as zeros).

The ragged mask (with the 1.1 scale baked in) rides along as a [128,256]
f32 input tile; each data tile is multiplied in place on DVE by a
step-0-broadcast slice of it (per (row, n) scalar broadcast over f=256).
K is rounded up to a multiple of 8 and maxed across cores, which only
adds rows whose mask is all-zero in that chunk (stored as zeros —
still exact).

Loads issue on the SP HWDGE ring, stores on the ACT ring, so a store
stalled on its mul never head-blocks later loads.
"""

import numpy as np

from contextlib import ExitStack

import concourse.bass as bass
import concourse.tile as tile
from concourse import bacc, mybir
from concourse import bass_utils

B, T, N, F = 32, 64, 128, 256
SCALE = 1.1  # ALPHA + BETA
N_CORES = 8
BPC = B // N_CORES            # batches per core = 4
BT = BPC * T                  # 256 flattened (b,t) rows per core
A = BT // 128                 # 2 partition-groups of rows
NSPLIT = 8                    # n-chunks per group
NPER = N // NSPLIT            # n-values per chunk
M = NPER * F                  # free elems per partition per tile
BUFS = 8
KQUANT = 8                    # round prefix counts up to a multiple of this

_CACHE = {}


NT = A * NSPLIT               # total tiles


def _bases(kpat):
    """Stagger each tile's SBUF base partition so the per-partition (and
    hence per-SDMA-engine) byte load is level, instead of every prefix
    hammering partitions 0..K."""
    order = sorted(range(NT), key=lambda t: -kpat[t])
    load = np.zeros(128, dtype=np.int64)
    bases = [0] * NT
    for t in order:
        k = kpat[t]
        if k == 0 or k == 128:
            load += (k == 128)
            continue
        best, bestkey = 0, None
        for b in range(0, 128 - k + 1, KQUANT):
            seg = load[b:b + k]
            key = (int(seg.max()), int(seg.sum()))
            if bestkey is None or key < bestkey:
                best, bestkey = b, key
        bases[t] = best
        load[best:best + k] += 1
    return tuple(bases)


def _build(kpat):
    """Build + compile the SPMD program for prefix-count pattern `kpat`
    (tuple of NT ints in [0,128], multiples of KQUANT)."""
    bases = _bases(kpat)
    nc = bacc.Bacc(
        "TRN2",
        target_bir_lowering=False,
        debug=False,
        enable_asserts=False,
        num_devices=N_CORES,
    )
    x = nc.dram_tensor("x", [A, 128, NSPLIT, M], mybir.dt.float32,
                       kind="ExternalInput").ap()
    mask = nc.dram_tensor("mask", [128, NT * NPER], mybir.dt.float32,
                          kind="ExternalInput").ap()
    y = nc.dram_tensor("y", [A, 128, NSPLIT, M], mybir.dt.float32,
                       kind="ExternalOutput").ap()

    with tile.TileContext(nc) as tc:
        with ExitStack() as ctx:
            mask_pool = ctx.enter_context(tc.tile_pool(name="maskp", bufs=1))
            pool = ctx.enter_context(tc.tile_pool(name="xp", bufs=BUFS))

            mask_sb = mask_pool.tile([128, NT * NPER], mybir.dt.float32,
                                     name="mask_sb")
            nc.scalar.dma_start(mask_sb[:], mask[:])

            for a in range(A):
                for s in range(NSPLIT):
                    ti = a * NSPLIT + s
                    K = kpat[ti]
                    if K == 0:
                        continue
                    b = bases[ti]
                    t = pool.tile([128, M], mybir.dt.float32, name="xt")
                    nc.sync.dma_start(t[b:b + K, :], x[a, 0:K, s, :])
                    # compute windows are quadrant-restricted; run the mul
                    # over all 128 partitions (extra lanes are free on DVE,
                    # see mask==0, and are never stored).
                    t3 = t.rearrange("p (n f) -> p n f", f=F)
                    msl = mask_sb[:, ti * NPER:(ti + 1) * NPER]
                    nc.vector.tensor_mul(
                        t3, t3, msl.unsqueeze(2).broadcast_to((128, NPER, F))
                    )
                    nc.scalar.dma_start(y[a, 0:K, s, :], t[b:b + K, :])

    nc.compile()
    return nc


def _get_nc(kpat):
    if kpat not in _CACHE:
        _CACHE[kpat] = _build(kpat)
    return _CACHE[kpat]


def _host_prep(input_tensor, batch_set_size):
    """Sort rows per core, build in_maps and the global K pattern."""
    ss_all = np.asarray(batch_set_size).reshape(B, T).astype(np.int64)
    n_idx = np.arange(N, dtype=np.int64)

    perms = []
    sorted_ss = []
    xs_sorted = []
    kmat = np.zeros((N_CORES, NT), dtype=np.int64)
    for i in range(N_CORES):
        ss = ss_all[i * BPC:(i + 1) * BPC].reshape(BT)
        perm = np.argsort(-ss, kind="stable")
        perms.append(perm)
        ss_sorted = ss[perm]
        sorted_ss.append(ss_sorted)

        xs = np.asarray(input_tensor[i * BPC:(i + 1) * BPC],
                        dtype=np.float32).reshape(BT, N * F)
        xs_sorted.append(np.ascontiguousarray(xs[perm]))

        for a in range(A):
            g = ss_sorted[a * 128:(a + 1) * 128]
            for s in range(NSPLIT):
                kmat[i, a * NSPLIT + s] = int((g > s * NPER).sum())

    kmax = kmat.max(axis=0)
    kpat = tuple(
        int(min(128, -(-k // KQUANT) * KQUANT)) for k in kmax
    )
    bases = _bases(kpat)

    in_maps = []
    for i in range(N_CORES):
        x_dev = xs_sorted[i].reshape(A, 128, NSPLIT, M)
        mrows = (n_idx[None, :] < sorted_ss[i][:, None]).astype(np.float32)
        mrows *= np.float32(SCALE)                           # [BT, N] sorted
        mdev = np.zeros((128, NT * NPER), dtype=np.float32)
        for a in range(A):
            for s in range(NSPLIT):
                ti = a * NSPLIT + s
                K, b = kpat[ti], bases[ti]
                if K == 0:
                    continue
                mdev[b:b + K, ti * NPER:(ti + 1) * NPER] = \
                    mrows[a * 128:a * 128 + K, s * NPER:(s + 1) * NPER]
        in_maps.append({"x": x_dev, "mask": mdev})
    return in_maps, perms, kpat


def kernel(input_tensor, batch_set_size):
    input_tensor = np.asarray(input_tensor, dtype=np.float32)
    in_maps, perms, kpat = _host_prep(input_tensor, batch_set_size)

    if all(k == 0 for k in kpat):
        return np.zeros((B, T, N, F), dtype=np.float32)

    nc = _get_nc(kpat)
    res = bass_utils.run_bass_kernel_spmd(
        nc, in_maps, core_ids=list(range(N_CORES))
    )
    out = np.empty((B, T, N, F), dtype=np.float32)
    for i in range(N_CORES):
        y_rows = res.results[i]["y"].reshape(BT, N * F)
        dst = out[i * BPC:(i + 1) * BPC].reshape(BT, N * F)
        dst[perms[i]] = y_rows
    return out
